# revision 10
# baseline (speedup 1.0000x reference)
"""Trainium2 Bass kernel for nn_Decoding_model (GL BCH(127,64) min-sum decoder).

Data-parallel over 8 NeuronCores: batch 256 -> 32 codewords/core.

Math reformulation: the reference's super-batch (3 views x 4 shifts = 12
permuted copies of each codeword) is absorbed into 12 column-permutation
"classes".  Each iteration, for each class c and base check m, the virtual
check operates on x[b, P_c(n')] for n' in supp(m).  Device layout packs
(class, batch) = 4x32 = 128 on partitions per class-tile (3 tiles).

Per iteration, per tile:
  XP   [n', (c,b)]  = permuted copies of x (4 tiny matmuls, fp32 one-hot)
  Ga   [(c,b), slot] = gather of |XP| by shared base-H selector (matmul)
  min1/min2 per check: segmented reduce_min / is_le / copy_predicated / reduce
  rs (row sign) via negative-count matmul + int parity
  temp = a1 @ H - scatter(C * a2)  accumulated in PSUM
         (a1 = rs*min1, a2 = rs*(min1-min2), scatter via chunk transpose+matmul)
  unpermute per class and accumulate over the 12 classes -> tempF [n, b]
  x <- x + softplus(cw)/12 * sign(x) * tempF
Loss (final iterate, row layout) computed on device; per-row partials output.
"""
import sys
import numpy as np

sys.path.insert(0, "/opt/trn_rl_repo")

M, N = 63, 127
B = 256
NCORES = 8
BS = B // NCORES          # 32
NUM_SHIFTS = 4
NITER = 3
BIG = 1e30
NBUCK = 4

_cache = {}
SKIP = set()


def _perm_classes():
    pos = (2 * np.arange(N)) % N
    w = (N + 1) // 2
    sig = np.zeros((3, N), np.int64)
    sig[0] = np.arange(N)
    sig[1, :w] = 2 * np.arange(w)
    sig[1, w:] = 2 * np.arange(N - w) + 1
    sig[2] = pos
    shifts = [i * (N // NUM_SHIFTS) for i in range(NUM_SHIFTS)]
    pcls = np.zeros((12, N), np.int64)
    ci = 0
    for s in shifts:
        for v in range(3):
            jj = (np.arange(N) - s) % N
            pcls[ci] = sig[v][jj]
            ci += 1
    return pcls


def _build_constants(H):
    """All selector matrices + layout metadata, from the actual H input."""
    pcls = _perm_classes()
    sup = [np.nonzero(H[m] != 0)[0] for m in range(M)]
    nnz = np.array([len(s) for s in sup])
    order = np.argsort(nnz, kind="stable")        # ascending nnz
    Hs_sup = [sup[m] for m in order]
    nnz_s = nnz[order]

    # buckets of contiguous checks with shared padded length
    bounds = [round(i * M / NBUCK) for i in range(NBUCK + 1)]
    buckets = []   # (col_off, n_checks, L)
    col = 0
    for bi in range(NBUCK):
        lo, hi = bounds[bi], bounds[bi + 1]
        ln = int(nnz_s[lo:hi].max())
        buckets.append((col, hi - lo, ln))
        col += (hi - lo) * ln
    WG = col
    NCH = (WG + 127) // 128
    WGpad = NCH * 128

    # slot -> (check j, l) mapping and base column of each real slot
    slot_n = np.full(WG, -1, np.int64)     # base col gathered by slot (-1 = pad)
    slot_j = np.full(WG, -1, np.int64)
    for bi, (off, nch, ln) in enumerate(buckets):
        j0 = bounds[bi]
        for jj in range(nch):
            j = j0 + jj
            s = Hs_sup[j]
            base = off + jj * ln
            slot_n[base:base + len(s)] = s
            slot_j[base:base + len(s)] = j

    # gather selector [128, WG]: row n' -> slot; pads select row 127 (BIG row)
    Gsel = np.zeros((128, WG), np.float32)
    for w in range(WG):
        Gsel[slot_n[w] if slot_n[w] >= 0 else 127, w] = 1.0

    # scatter chunks [128, 127] with -1 at [slot_row, base_col]
    NScat = np.zeros((NCH, 128, N), np.float32)
    for w in range(WG):
        if slot_n[w] >= 0:
            NScat[w // 128, w % 128, slot_n[w]] = -1.0

    # H in check-sorted order for term1 / cnt
    Hs = (H[order] != 0).astype(np.float32)        # [63, 127]
    HsupT = np.zeros((128, M), np.float32)
    HsupT[:N, :] = Hs.T

    # per-class permutation selectors
    SelP = np.zeros((12, 128, N), np.float32)      # [n, n'] 1 iff n == P_c(n')
    USel = np.zeros((12, 128, N), np.float32)      # [n', k] 1 iff k == P_c(n')
    for c in range(12):
        for npr in range(N):
            SelP[c, pcls[c][npr], npr] = 1.0
            USel[c, npr, pcls[c][npr]] = 1.0

    ident = np.eye(128, dtype=np.float32)

    return dict(Gsel=Gsel, NScat=NScat, Hs=Hs, HsupT=HsupT, SelP=SelP,
                USel=USel, ident=ident, buckets=buckets, WG=WG, NCH=NCH,
                WGpad=WGpad, order=order)


def _pack_blob(C):
    """Column layout of the [128, CW] per-core input blob (fp32)."""
    cols = {}
    parts = []
    off = 0

    def add(name, arr2d):
        nonlocal off
        w = arr2d.shape[1]
        a = np.zeros((128, w), np.float32)
        a[:arr2d.shape[0], :] = arr2d
        cols[name] = (off, w)
        parts.append(a)
        off += w

    add("ident", C["ident"])
    for c in range(12):
        add(f"selp{c}", C["SelP"][c])
        add(f"usel{c}", C["USel"][c])
    add("gsel", C["Gsel"])
    for k in range(C["NCH"]):
        add(f"nscat{k}", C["NScat"][k])
    add("h63", C["Hs"])                      # rows 0:63
    add("hsupt", C["HsupT"])
    add("xT", np.zeros((127, BS), np.float32))      # filled per core
    add("lab", np.zeros((BS, N), np.float32))       # int32 bits per core
    add("cw", np.zeros((1, 1), np.float32))
    blob = np.concatenate(parts, axis=1)
    return blob, cols


def _build_program(C, cols, CW):
    import concourse.bass as bass
    import concourse.bacc as bacc
    import concourse.tile as tile
    from concourse import mybir
    from contextlib import ExitStack

    AF = mybir.ActivationFunctionType
    AL = mybir.AluOpType
    f32 = mybir.dt.float32
    buckets, WG, NCH, WGpad = C["buckets"], C["WG"], C["NCH"], C["WGpad"]

    nc = bacc.Bacc("TRN2", target_bir_lowering=False, debug=False)
    blob_d = nc.dram_tensor("blob", [128, CW], f32, kind="ExternalInput")
    out_d = nc.dram_tensor("out", [BS, 3 * N + 1], f32, kind="ExternalOutput")

    with tile.TileContext(nc) as tc, ExitStack() as ctx:
        cons = ctx.enter_context(tc.tile_pool(name="cons", bufs=1))
        sb = ctx.enter_context(tc.tile_pool(name="sb", bufs=2))
        sga = ctx.enter_context(tc.tile_pool(name="sga", bufs=2))
        sxt = ctx.enter_context(tc.tile_pool(name="sxt", bufs=2))
        sout = ctx.enter_context(tc.tile_pool(name="sout", bufs=1))
        psA = ctx.enter_context(tc.tile_pool(name="psA", bufs=2, space="PSUM"))
        psB = ctx.enter_context(tc.tile_pool(name="psB", bufs=2, space="PSUM"))
        psT = ctx.enter_context(tc.tile_pool(name="psT", bufs=1, space="PSUM"))
        psF = ctx.enter_context(tc.tile_pool(name="psF", bufs=1, space="PSUM"))

        blob = cons.tile([128, CW], f32)
        nc.gpsimd.dma_start(blob[:], blob_d[:, :])

        def cslice(name):
            o, w = cols[name]
            return blob[:, o:o + w]

        ident = cslice("ident")
        gsel = cslice("gsel")
        h63 = cslice("h63")
        hsupt = cslice("hsupt")
        xT0 = cslice("xT")
        labf32bits = cslice("lab")
        cw = cslice("cw")

        outb = sout.tile([BS, 3 * N + 1], f32)

        # log1p(exp(u)) for u<=0: w=e^u; t0=w(w+2)/(2w+2); 3x Newton
        # t <- t - 1 + (1+w)e^-t.  Emitted on arbitrary [P,F] tiles.
        def emit_log1p_exp(src, dst, shape, tg):
            P_, F_ = shape
            w = sb.tile([P_, F_], f32, tag=tg + "w")
            nc.scalar.activation(w[:], src, AF.Exp)
            wp2 = sb.tile([P_, F_], f32, tag=tg + "a")
            nc.vector.tensor_scalar(out=wp2[:], in0=w[:], scalar1=2.0,
                                    scalar2=None, op0=AL.add)
            num = sb.tile([P_, F_], f32, tag=tg + "b")
            nc.vector.tensor_tensor(out=num[:], in0=w[:], in1=wp2[:],
                                    op=AL.mult)
            den = sb.tile([P_, F_], f32, tag=tg + "c")
            nc.vector.tensor_scalar(out=den[:], in0=w[:], scalar1=2.0,
                                    scalar2=2.0, op0=AL.mult, op1=AL.add)
            rden = sb.tile([P_, F_], f32, tag=tg + "d")
            nc.vector.reciprocal(rden[:], den[:])
            t = sb.tile([P_, F_], f32, tag=tg + "t")
            nc.vector.tensor_tensor(out=t[:], in0=num[:], in1=rden[:],
                                    op=AL.mult)
            wp1 = wp2  # reuse: w + 1
            nc.vector.tensor_scalar(out=wp1[:], in0=w[:], scalar1=1.0,
                                    scalar2=None, op0=AL.add)
            for _ in range(3):
                e = sb.tile([P_, F_], f32, tag=tg + "e")
                nc.scalar.activation(e[:], t[:], AF.Exp, scale=-1.0)
                q = sb.tile([P_, F_], f32, tag=tg + "q")
                nc.vector.tensor_tensor(out=q[:], in0=wp1[:], in1=e[:],
                                        op=AL.mult)
                nc.vector.tensor_scalar(out=q[:], in0=q[:], scalar1=1.0,
                                        scalar2=-1.0, op0=AL.mult, op1=AL.add)
                nc.vector.tensor_tensor(out=t[:], in0=t[:], in1=q[:],
                                        op=AL.add)
            nc.vector.tensor_copy(dst, t[:])

        # softplus(cw) = relu(cw) + log1p(exp(-|cw|)) on a [1,1] tile
        spn = sb.tile([1, 1], f32, tag="spn")
        nc.vector.tensor_scalar(out=spn[:], in0=cw[0:1, 0:1], scalar1=-1.0,
                                scalar2=None, op0=AL.mult)
        nc.vector.tensor_tensor(out=spn[:], in0=spn[:], in1=cw[0:1, 0:1],
                                op=AL.min)                      # -|cw|
        sp1 = sb.tile([1, 1], f32, tag="sp1")
        emit_log1p_exp(spn[:], sp1[:], (1, 1), "sp")
        spr = sb.tile([1, 1], f32, tag="spr")
        nc.vector.tensor_scalar(out=spr[:], in0=cw[0:1, 0:1], scalar1=0.0,
                                scalar2=None, op0=AL.max)       # relu(cw)
        nc.vector.tensor_tensor(out=sp1[:], in0=sp1[:], in1=spr[:], op=AL.add)
        spp = psF.tile([128, 2], f32)
        ones_row = sb.tile([1, 128], f32)
        nc.vector.memset(ones_row[:], 1.0)
        nc.tensor.matmul(spp[:, 0:1], lhsT=ones_row[0:1, :], rhs=sp1[0:1, 0:1],
                         start=True, stop=True)
        sp_col = sb.tile([128, 1], f32)
        nc.scalar.copy(sp_col[:], spp[:, 0:1])
        bigc = sb.tile([128, 1], f32, tag="bigc")
        nc.vector.memset(bigc[:], BIG)

        xT = sxt.tile([128, BS], f32)
        nc.vector.tensor_copy(xT[0:N, :], xT0[0:N, :])

        for it in range(NITER):
            tempFp = psF.tile([128, BS], f32, tag="tempF")
            for t in range(3):
                # --- permuted copies XP [n' , (c,b)] ---
                XPp = psA.tile([128, 128], f32, tag="small")
                for c in range(4):
                    nc.tensor.matmul(
                        XPp[0:N, c * BS:(c + 1) * BS],
                        lhsT=cslice(f"selp{4 * t + c}")[0:N, 0:N],
                        rhs=xT[0:N, :], start=True, stop=True)
                XP = sb.tile([128, 128], f32, tag="XP")
                nc.vector.memset(XP[:], BIG)
                nc.scalar.copy(XP[0:N, :], XPp[0:N, :])
                AbsXP = sb.tile([128, 128], f32, tag="AbsXP")
                nc.vector.tensor_scalar(out=AbsXP[:], in0=XP[:], scalar1=-1.0,
                                        scalar2=None, op0=AL.mult)
                nc.vector.tensor_tensor(out=AbsXP[:], in0=AbsXP[:], in1=XP[:],
                                        op=AL.max)
                NegXP = sb.tile([128, 128], f32, tag="NegXP")
                nc.vector.tensor_scalar(out=NegXP[:], in0=XP[:], scalar1=0.0,
                                        scalar2=None, op0=AL.is_lt)

                # --- gather |x| per check slot ---
                Ga = sga.tile([128, WGpad], f32, tag="Ga")
                for k0 in range(0, WG, 512):
                    k1 = min(k0 + 512, WG)
                    Gp = psB.tile([128, 512], f32, tag="gat")
                    nc.tensor.matmul(Gp[:, 0:k1 - k0], lhsT=AbsXP[:],
                                     rhs=gsel[:, k0:k1], start=True, stop=True)
                    nc.scalar.copy(Ga[:, k0:k1], Gp[:, 0:k1 - k0])
                if WGpad > WG:
                    nc.vector.memset(Ga[:, WG:WGpad], 0.0)

                # --- negative-count -> row sign ---
                cntp = psT.tile([128, M], f32, tag="cnt")
                nc.tensor.matmul(cntp[:], lhsT=NegXP[:], rhs=hsupt[:, 0:M],
                                 start=True, stop=True)

                # --- min1 / min2 per check (bucketed segmented ops) ---
                min1 = sb.tile([128, M], f32, tag="min1")
                min2 = sb.tile([128, M], f32, tag="min2")
                Cm = sga.tile([128, WGpad], mybir.dt.uint8, tag="Cm")
                for bi, (off, nch, ln) in enumerate(buckets):
                    jlo = sum(b[1] for b in buckets[:bi])
                    g3 = Ga[:, off:off + nch * ln].rearrange(
                        "p (s l) -> p s l", s=nch)
                    m1s = min1[:, jlo:jlo + nch]
                    nc.vector.tensor_reduce(m1s, g3, axis=mybir.AxisListType.X,
                                            op=AL.min)
                    m1bc = m1s.unsqueeze(2).broadcast_to([128, nch, ln])
                    c3 = Cm[:, off:off + nch * ln].rearrange(
                        "p (s l) -> p s l", s=nch)
                    nc.vector.tensor_tensor(out=c3, in0=g3, in1=m1bc,
                                            op=AL.is_le)
                    nc.vector.copy_predicated(
                        out=g3, mask=c3,
                        data=bigc[:, 0:1].unsqueeze(2)
                        .broadcast_to([128, nch, ln]))
                    nc.vector.tensor_reduce(min2[:, jlo:jlo + nch], g3,
                                            axis=mybir.AxisListType.X, op=AL.min)

                # row sign via parity
                cnti = sb.tile([128, M], mybir.dt.int32, tag="cnti")
                nc.vector.tensor_copy(cnti[:], cntp[:])
                pari = sb.tile([128, M], mybir.dt.int32, tag="pari")
                nc.vector.tensor_scalar(out=pari[:], in0=cnti[:], scalar1=1,
                                        scalar2=None, op0=AL.bitwise_and)
                rs = sb.tile([128, M], f32, tag="rs")
                nc.vector.tensor_copy(rs[:], pari[:])
                nc.vector.tensor_scalar(out=rs[:], in0=rs[:], scalar1=-2.0,
                                        scalar2=1.0, op0=AL.mult, op1=AL.add)

                a1 = sb.tile([128, M], f32, tag="a1")
                nc.vector.tensor_tensor(out=a1[:], in0=rs[:], in1=min1[:],
                                        op=AL.mult)
                d12 = sb.tile([128, M], f32, tag="d12")
                nc.vector.tensor_tensor(out=d12[:], in0=min1[:], in1=min2[:],
                                        op=AL.subtract)
                a2 = sb.tile([128, M], f32, tag="a2")
                nc.vector.tensor_tensor(out=a2[:], in0=rs[:], in1=d12[:],
                                        op=AL.mult)

                # W = C * a2_bc
                W = sga.tile([128, WGpad], f32, tag="W")
                for bi, (off, nch, ln) in enumerate(buckets):
                    jlo = sum(b[1] for b in buckets[:bi])
                    c3 = Cm[:, off:off + nch * ln].rearrange(
                        "p (s l) -> p s l", s=nch)
                    a2bc = a2[:, jlo:jlo + nch].unsqueeze(2).broadcast_to(
                        [128, nch, ln])
                    w3 = W[:, off:off + nch * ln].rearrange(
                        "p (s l) -> p s l", s=nch)
                    nc.vector.tensor_tensor(out=w3, in0=c3, in1=a2bc,
                                            op=AL.mult)
                if WGpad > WG:
                    nc.vector.memset(W[:, WG:WGpad], 0.0)

                # --- temp in class coords: term1 - scatter(W) ---
                a1Tp = psA.tile([128, 128], f32, tag="small")
                nc.tensor.matmul(a1Tp[0:M, :], lhsT=a1[:], rhs=ident[:, :],
                                 is_transpose=True, start=True, stop=True)
                a1T = sb.tile([128, 128], f32, tag="a1T")
                nc.scalar.copy(a1T[0:M, :], a1Tp[0:M, :])
                T2p = psT.tile([128, N], f32, tag="T2")
                nc.tensor.matmul(T2p[:], lhsT=a1T[0:M, :], rhs=h63[0:M, 0:N],
                                 start=True, stop=False)
                for k in range(NCH if "scat" not in SKIP else 0):
                    WTp = psA.tile([128, 128], f32, tag="small")
                    nc.tensor.matmul(WTp[:], lhsT=W[:, k * 128:(k + 1) * 128],
                                     rhs=ident[:, :], is_transpose=True,
                                     start=True, stop=True)
                    WT = sb.tile([128, 128], f32, tag="WT")
                    nc.any.tensor_copy(WT[:], WTp[:])
                    nc.tensor.matmul(T2p[:], lhsT=WT[:],
                                     rhs=cslice(f"nscat{k}")[:, 0:N],
                                     start=False,
                                     stop=(k == NCH - 1 and "scat" not in SKIP)
                                     or ("scat" in SKIP))

                T2s = sb.tile([128, N], f32, tag="T2s")
                nc.scalar.copy(T2s[:], T2p[:])
                T2Tp = psA.tile([128, 128], f32, tag="small")
                nc.tensor.matmul(T2Tp[0:N, :], lhsT=T2s[:], rhs=ident[:, :],
                                 is_transpose=True, start=True, stop=True)
                T2T = sb.tile([128, 128], f32, tag="T2T")
                nc.scalar.copy(T2T[0:N, :], T2Tp[0:N, :])

                for c in range(4):
                    nc.tensor.matmul(
                        tempFp[0:N, :], lhsT=cslice(f"usel{4 * t + c}")[0:N, 0:N],
                        rhs=T2T[0:N, c * BS:(c + 1) * BS],
                        start=(t == 0 and c == 0), stop=(t == 2 and c == 3))

            # --- finale: x += sp/12 * sign(x) * tempF ---
            sgT = sb.tile([128, BS], f32, tag="sgT")
            sgN = sb.tile([128, BS], f32, tag="sgN")
            nc.vector.tensor_scalar(out=sgT[0:N, :], in0=xT[0:N, :],
                                    scalar1=0.0, scalar2=None, op0=AL.is_gt)
            nc.vector.tensor_scalar(out=sgN[0:N, :], in0=xT[0:N, :],
                                    scalar1=0.0, scalar2=None, op0=AL.is_lt)
            nc.vector.tensor_tensor(out=sgT[0:N, :], in0=sgT[0:N, :],
                                    in1=sgN[0:N, :], op=AL.subtract)
            u = sb.tile([128, BS], f32, tag="u")
            nc.vector.tensor_tensor(out=u[0:N, :], in0=tempFp[0:N, :],
                                    in1=sgT[0:N, :], op=AL.mult)
            nc.vector.tensor_scalar(out=u[0:N, :], in0=u[0:N, :],
                                    scalar1=sp_col[0:N, 0:1],
                                    scalar2=1.0 / 12.0, op0=AL.mult,
                                    op1=AL.mult)
            xTn = sxt.tile([128, BS], f32)
            nc.vector.tensor_tensor(out=xTn[0:N, :], in0=xT[0:N, :],
                                    in1=u[0:N, :], op=AL.add)
            xT = xTn

            # --- iterate output in row layout ---
            oTp = psB.tile([128, 512], f32, tag="gat")
            nc.tensor.matmul(oTp[0:BS, 0:N], lhsT=xT[0:N, :],
                             rhs=ident[0:N, 0:N], is_transpose=True,
                             start=True, stop=True)
            nc.scalar.copy(outb[:, it * N:(it + 1) * N], oTp[0:BS, 0:N])

        # --- loss on final iterate (row layout) ---
        x3 = outb[:, 2 * N:3 * N]
        labf = sb.tile([BS, N], f32, tag="labf")
        nc.vector.tensor_copy(labf[:], labf32bits[0:BS, :]
                              .bitcast(mybir.dt.int32))
        z = sb.tile([BS, N], f32, tag="z")
        nc.vector.tensor_scalar(out=z[:], in0=x3, scalar1=-1.0, scalar2=None,
                                op0=AL.mult)
        mz = sb.tile([BS, N], f32, tag="mz")
        nc.vector.tensor_scalar(out=mz[:], in0=z[:], scalar1=0.0, scalar2=None,
                                op0=AL.max)
        zy = sb.tile([BS, N], f32, tag="zy")
        nc.vector.tensor_tensor(out=zy[:], in0=z[:], in1=labf[:], op=AL.mult)
        az = sb.tile([BS, N], f32, tag="az")
        nc.vector.tensor_scalar(out=az[:], in0=z[:], scalar1=-1.0,
                                scalar2=None, op0=AL.mult)
        nc.vector.tensor_tensor(out=az[:], in0=az[:], in1=z[:], op=AL.min)
        spz = sb.tile([BS, N], f32, tag="spz")
        emit_log1p_exp(az[:], spz[:], (BS, N), "lz")
        ce = sb.tile([BS, N], f32, tag="ce")
        nc.vector.tensor_tensor(out=ce[:], in0=mz[:], in1=zy[:], op=AL.subtract)
        nc.vector.tensor_tensor(out=ce[:], in0=ce[:], in1=spz[:], op=AL.add)
        sgn = sb.tile([BS, N], f32, tag="sgn")
        sgn2 = sb.tile([BS, N], f32, tag="sgn2")
        nc.vector.tensor_scalar(out=sgn[:], in0=x3, scalar1=0.0,
                                scalar2=None, op0=AL.is_gt)
        nc.vector.tensor_scalar(out=sgn2[:], in0=x3, scalar1=0.0,
                                scalar2=None, op0=AL.is_lt)
        nc.vector.tensor_tensor(out=sgn[:], in0=sgn[:], in1=sgn2[:],
                                op=AL.subtract)
        tgt = sb.tile([BS, N], f32, tag="tgt")
        nc.vector.tensor_scalar(out=tgt[:], in0=labf[:], scalar1=-2.0,
                                scalar2=1.0, op0=AL.mult, op1=AL.add)
        ne = sb.tile([BS, N], f32, tag="ne")
        nc.vector.tensor_tensor(out=ne[:], in0=sgn[:], in1=tgt[:],
                                op=AL.not_equal)
        nc.vector.tensor_scalar(out=ne[:], in0=ne[:], scalar1=1.0, scalar2=None,
                                op0=AL.add)
        nc.vector.tensor_tensor(out=ce[:], in0=ce[:], in1=ne[:], op=AL.mult)
        nc.vector.tensor_reduce(outb[:, 3 * N:3 * N + 1], ce[:],
                                axis=mybir.AxisListType.X, op=AL.add)

        nc.gpsimd.dma_start(out_d[:, :], outb[:])

    nc.compile()
    return nc


def kernel(soft_input, labels, H, check_weight):
    from concourse.bass_utils import run_bass_kernel_spmd

    key = (H.tobytes(),)
    if _cache.get("key") != key:
        C = _build_constants(np.asarray(H))
        blob, cols = _pack_blob(C)
        nc = _build_program(C, cols, blob.shape[1])
        _cache.update(key=key, C=C, blob=blob, cols=cols, nc=nc)

    C, blob, cols, nc = _cache["C"], _cache["blob"], _cache["cols"], _cache["nc"]

    soft = np.ascontiguousarray(np.asarray(soft_input, np.float32))
    lab = np.ascontiguousarray(np.asarray(labels, np.int32))
    cwv = np.asarray(check_weight, np.float32).reshape(1)

    in_maps = []
    for core in range(NCORES):
        b = blob.copy()
        o, w = cols["xT"]
        b[0:N, o:o + BS] = soft[core * BS:(core + 1) * BS].T
        o, w = cols["lab"]
        b[0:BS, o:o + N] = lab[core * BS:(core + 1) * BS].view(np.float32)
        o, w = cols["cw"]
        b[0, o] = cwv[0]
        in_maps.append({"blob": b})

    res = run_bass_kernel_spmd(nc, in_maps, core_ids=list(range(NCORES)))

    outs = np.zeros((NITER + 1, B, N), np.float32)
    outs[0] = soft
    loss = np.float64(0.0)
    for core in range(NCORES):
        o = res.results[core]["out"]
        for it in range(NITER):
            outs[it + 1, core * BS:(core + 1) * BS] = o[:, it * N:(it + 1) * N]
        loss += np.sum(o[:, 3 * N].astype(np.float64))
    return outs, np.float32(loss)
